# revision 4
# baseline (speedup 1.0000x reference)
"""GCN message-passing kernel for 8 trn2 NeuronCores (bass/Tile).

Sharding: nodes are degree-sorted and dealt round-robin across 8 cores
(graph-parallel, dst-sharded).  Each core computes t = dis*(h @ W') for
its node shard; two AllGathers replicate the scaled table in bf16 (a lo
half and an overlapping hi half so every gather index fits in int16);
each core then fetches its in-edge source rows with bulk SWDGE
dma_gather instructions (4 queues) and reduces them with fused
scalar_tensor_tensor multiply-accumulates on the Vector and GpSimd
engines.  BatchNorm (eval mode) is folded into the weights/offsets on
the host; self-loops are applied from the local table copy without any
gather.
"""

import sys

sys.path.insert(0, "/opt/trn_rl_repo")

import numpy as np
import ml_dtypes

import concourse.bass as bass
import concourse.bacc as bacc
import concourse.mybir as mybir
from concourse.bass_utils import run_bass_kernel_spmd
from concourse.masks import make_identity
from concourse.tile import TileContext

N = 50000
E = 800000
CIN = 128
CH = 128
COUT = 64
EPS = 1e-5
NCORES = 8
P = 128
SHARD = 6272          # 49 blocks * 128
NBLK = SHARD // P     # 49
AGL = 4096            # lo AllGather rows per core  (locals [0, 4096))
FLEX0 = 2176          # hi AllGather covers locals [2176, 6272)
AGH = 4096
LO_BLOCKS = AGL // P      # 32
HI_BLK0 = FLEX0 // P      # 17
CHUNK_COLS = 48           # gather chunk: 48 cols = 6144 indices
NQ = 4                    # SWDGE queues

F32 = mybir.dt.float32
BF16 = mybir.dt.bfloat16
I16 = mybir.dt.int16


def _host_prep(x, edge_index, edge_weights):
    """Shard nodes, build the per-core gather/slot layout (pure numpy)."""
    src = edge_index[0].astype(np.int64)
    dst = edge_index[1].astype(np.int64)
    ew = edge_weights.astype(np.float64)

    degc = np.bincount(dst, minlength=N)
    wdeg = np.bincount(dst, weights=ew, minlength=N) + 1.0
    dis = (1.0 / np.sqrt(wdeg)).astype(np.float32)

    order = np.argsort(degc, kind="stable")
    rank = np.empty(N, np.int64)
    rank[order] = np.arange(N)
    core_of = rank % NCORES
    local_of = rank // NCORES

    lo_row = core_of * AGL + local_of                 # valid iff local < AGL
    hi_row = core_of * AGH + (local_of - FLEX0)       # valid iff local >= FLEX0

    # edge category by src position
    s_local = local_of[src]
    cat = np.where(s_local < FLEX0, 0, np.where(s_local < AGL, 1, 2))

    d_core = core_of[dst]
    d_local = local_of[dst]
    d_blk = d_local // P

    # global (cross-core) per-node counts -> per-block uniform widths
    gl_node = rank[dst]  # global rank of dst
    a_cnt = np.bincount(gl_node[cat == 0], minlength=N)
    f_cnt = np.bincount(gl_node[cat == 1], minlength=N)
    b_cnt = np.bincount(gl_node[cat == 2], minlength=N)
    d_cnt = a_cnt + f_cnt + b_cnt
    # block j holds global ranks [j*1024, (j+1)*1024)
    pad_to = NBLK * P * NCORES  # 50176 > N: pad counts with zeros
    def _blkmax(cnt):
        c = np.zeros(pad_to, cnt.dtype)
        c[:N] = cnt
        return c.reshape(NBLK, P * NCORES).max(axis=1)
    A_j = _blkmax(a_cnt)
    B_j = _blkmax(b_cnt)
    D_j = _blkmax(d_cnt)
    S_j = np.maximum(D_j, A_j + B_j)
    Whi = np.maximum(B_j, 1)
    Wlo = np.maximum(S_j - Whi, 1)

    clo = np.concatenate([[0], np.cumsum(Wlo)]).astype(np.int64)
    chi = np.concatenate([[0], np.cumsum(Whi)]).astype(np.int64)
    SLo, SHi = int(clo[-1]), int(chi[-1])

    # per-node lo quota: lo_p = max(a_p, d_p - Whi[blk])  (global, same formula
    # per core since widths are global maxima)
    blk_of_rank = np.minimum(np.arange(N) // (P * NCORES), NBLK - 1)
    lo_quota = np.maximum(a_cnt, d_cnt - Whi[blk_of_rank])

    # assign flex edges: per dst node, first (lo_quota - a) flex edges -> lo
    eidx = np.arange(E)
    flex_mask = cat == 1
    fe = eidx[flex_mask]
    fe_sorted = fe[np.argsort(gl_node[fe], kind="stable")]
    gn_f = gl_node[fe_sorted]
    starts = np.searchsorted(gn_f, np.arange(N))
    rank_in_node = np.arange(len(fe_sorted)) - starts[gn_f]
    goes_lo = rank_in_node < (lo_quota[gn_f] - a_cnt[gn_f])
    is_lo = np.zeros(E, bool)
    is_lo[cat == 0] = True
    is_lo[fe_sorted[goes_lo]] = True

    # slot index within (dst node, phase)
    def slots_for(mask):
        ee = eidx[mask]
        ee = ee[np.argsort(gl_node[ee], kind="stable")]
        gn = gl_node[ee]
        st = np.searchsorted(gn, np.arange(N))
        sl = np.arange(len(ee)) - st[gn]
        return ee, sl

    ee_lo, sl_lo = slots_for(is_lo)
    ee_hi, sl_hi = slots_for(~is_lo)

    # per-core arrays
    idx_lo = np.zeros((NCORES, SLo * P), np.int16)
    idx_hi = np.zeros((NCORES, SHi * P), np.int16)
    w_lo = np.zeros((NCORES, P, SLo), np.float32)
    w_hi = np.zeros((NCORES, P, SHi), np.float32)

    for arr_i, arr_w, ee, sl, row_of, cbase in (
        (idx_lo, w_lo, ee_lo, sl_lo, lo_row, clo),
        (idx_hi, w_hi, ee_hi, sl_hi, hi_row, chi),
    ):
        c = d_core[ee]
        p = d_local[ee] % P
        col = cbase[d_blk[ee]] + sl
        pos = col * P + p
        arr_i[c, pos] = row_of[src[ee]].astype(np.int16)
        arr_w[c, p, col] = (ew[ee] * dis[dst[ee]]).astype(np.float32)

    def wrap_idx(a):
        # gather wrapped layout: idx i -> [i%16, i//16], replicated 8x
        w16 = a.reshape(-1, 16).T.copy()
        return np.ascontiguousarray(np.tile(w16, (8, 1)))

    idx_lo_t = np.stack([wrap_idx(idx_lo[c]) for c in range(NCORES)])
    idx_hi_t = np.stack([wrap_idx(idx_hi[c]) for c in range(NCORES)])

    # per-core x shard and dis
    x_sh = np.zeros((NCORES, SHARD, CIN), np.float32)
    dis_sh = np.ones((NCORES, P, NBLK), np.float32)
    for c in range(NCORES):
        ranks = np.arange(c, N, NCORES)
        loc = ranks // NCORES
        x_sh[c, loc] = x[order[ranks]]
        dflat = np.ones(SHARD, np.float32)
        dflat[loc] = dis[order[ranks]]
        dis_sh[c] = dflat.reshape(NBLK, P).T

    return dict(
        order=order,
        dis=dis,
        Wlo=Wlo.astype(int),
        Whi=Whi.astype(int),
        clo=clo,
        chi=chi,
        SLo=SLo,
        SHi=SHi,
        idx_lo=idx_lo_t,
        idx_hi=idx_hi_t,
        w_lo=w_lo,
        w_hi=w_hi,
        x_sh=x_sh,
        dis_sh=dis_sh,
        # for host-side simulation/testing
        raw_idx_lo=idx_lo,
        raw_idx_hi=idx_hi,
    )


def _fold_params(inputs):
    """Host-side BN folding: per layer ell: W' = W * s_g[None,:], offset
    o = (cb - m) * s_g + b."""
    out = {}
    for ell in range(3):
        if ell == 0:
            W = np.asarray(inputs["w1"], np.float32)
            cb = np.asarray(inputs["b1"], np.float32)
            g = np.asarray(inputs["bn1_g"], np.float32)
            b = np.asarray(inputs["bn1_b"], np.float32)
            m = np.asarray(inputs["bn1_m"], np.float32)
            v = np.asarray(inputs["bn1_v"], np.float32)
        else:
            W = np.asarray(inputs["conv_ws"], np.float32)[ell - 1]
            cb = np.asarray(inputs["conv_bs"], np.float32)[ell - 1]
            g = np.asarray(inputs["bns_g"], np.float32)[ell - 1]
            b = np.asarray(inputs["bns_b"], np.float32)[ell - 1]
            m = np.asarray(inputs["bns_m"], np.float32)[ell - 1]
            v = np.asarray(inputs["bns_v"], np.float32)[ell - 1]
        s_g = g / np.sqrt(v + EPS)
        out[f"Wp{ell}"] = np.ascontiguousarray(W * s_g[None, :])
        out[f"o{ell}"] = (cb - m) * s_g + b
    return out


def _build_program(prep):
    Wlo, Whi, clo, chi = prep["Wlo"], prep["Whi"], prep["clo"], prep["chi"]
    SLo, SHi = prep["SLo"], prep["SHi"]

    nc = bacc.Bacc(num_swdge_queues=NQ)

    x_ext = nc.declare_dram_parameter("x", [SHARD, CIN], F32, isOutput=False)
    ilo_ext = nc.declare_dram_parameter("idx_lo", [P, SLo * 8], I16, isOutput=False)
    ihi_ext = nc.declare_dram_parameter("idx_hi", [P, SHi * 8], I16, isOutput=False)
    wlo_ext = nc.declare_dram_parameter("w_lo", [P, SLo], F32, isOutput=False)
    whi_ext = nc.declare_dram_parameter("w_hi", [P, SHi], F32, isOutput=False)
    dis_ext = nc.declare_dram_parameter("dis", [P, NBLK], F32, isOutput=False)
    wp_ext = [
        nc.declare_dram_parameter(f"Wp{ell}", [CH, CH], F32, isOutput=False)
        for ell in range(3)
    ]
    o_ext = [
        nc.declare_dram_parameter(f"o{ell}", [P, CH], F32, isOutput=False)
        for ell in range(3)
    ]
    lin1_ext = nc.declare_dram_parameter("lin1", [CH, CH], F32, isOutput=False)
    lin2_ext = nc.declare_dram_parameter("lin2", [CH, COUT], F32, isOutput=False)
    l1b_ext = nc.declare_dram_parameter("l1b", [P, CH], F32, isOutput=False)
    l2b_ext = nc.declare_dram_parameter("l2b", [P, COUT], F32, isOutput=False)
    y_ext = nc.declare_dram_parameter("y", [SHARD, COUT], F32, isOutput=True)

    lrelu = mybir.ActivationFunctionType.Lrelu
    actcopy = mybir.ActivationFunctionType.Copy

    with TileContext(nc) as tc:
        with (
            tc.tile_pool(name="const", bufs=1) as constp,
            tc.tile_pool(name="gpool", bufs=3) as gp,
            tc.tile_pool(name="work", bufs=4) as wk,
            tc.tile_pool(name="psum", bufs=2, space="PSUM") as pp,
            tc.tile_pool(name="psum2", bufs=2, space="PSUM") as pp2,
            tc.tile_pool(name="dram", bufs=1, space="DRAM") as dp,
        ):
            # ---- persistent SBUF ----
            ident = constp.tile([P, P], F32)
            make_identity(nc, ident[:])
            idx_lo_t = constp.tile([P, SLo * 8], I16)
            nc.sync.dma_start(out=idx_lo_t[:], in_=ilo_ext[:])
            idx_hi_t = constp.tile([P, SHi * 8], I16)
            nc.sync.dma_start(out=idx_hi_t[:], in_=ihi_ext[:])
            w_lo_t = constp.tile([P, SLo], F32)
            nc.sync.dma_start(out=w_lo_t[:], in_=wlo_ext[:])
            w_hi_t = constp.tile([P, SHi], F32)
            nc.sync.dma_start(out=w_hi_t[:], in_=whi_ext[:])
            dis_t = constp.tile([P, NBLK], F32)
            nc.sync.dma_start(out=dis_t[:], in_=dis_ext[:])
            Wp = []
            o_rep = []
            for ell in range(3):
                t = constp.tile([P, CH], F32, name=f"Wp{ell}")
                nc.sync.dma_start(out=t[:], in_=wp_ext[ell][:])
                Wp.append(t)
                t2 = constp.tile([P, CH], F32, name=f"o{ell}")
                nc.sync.dma_start(out=t2[:], in_=o_ext[ell][:])
                o_rep.append(t2)
            lin1_t = constp.tile([P, CH], F32)
            nc.sync.dma_start(out=lin1_t[:], in_=lin1_ext[:])
            lin2_t = constp.tile([P, COUT], F32)
            nc.sync.dma_start(out=lin2_t[:], in_=lin2_ext[:])
            l1b_t = constp.tile([P, CH], F32)
            nc.sync.dma_start(out=l1b_t[:], in_=l1b_ext[:])
            l2b_t = constp.tile([P, COUT], F32)
            nc.sync.dma_start(out=l2b_t[:], in_=l2b_ext[:])

            h = constp.tile([P, NBLK * CH], F32)
            for j in range(NBLK):
                nc.sync.dma_start(
                    out=h[:, j * CH : (j + 1) * CH],
                    in_=x_ext[j * P : (j + 1) * P, :],
                )
            accs = constp.tile([P, NBLK * CH], F32)
            tloc = constp.tile([P, NBLK * CH], BF16)

            # ---- DRAM intermediates ----
            agin_lo, agin_hi, tab_lo, tab_hi = [], [], [], []
            for ell in range(3):
                agin_lo.append(dp.tile([AGL, CH], BF16, name=f"aglo{ell}"))
                agin_hi.append(dp.tile([AGH, CH], BF16, name=f"aghi{ell}"))
                tab_lo.append(
                    dp.tile([NCORES * AGL, CH], BF16, name=f"tlo{ell}", addr_space="Shared")
                )
                tab_hi.append(
                    dp.tile([NCORES * AGH, CH], BF16, name=f"thi{ell}", addr_space="Shared")
                )

            def is_gps_block(j):
                return j % 4 == 3

            def emit_table(ell, j):
                """tloc[:, j] = dis * (h_j @ Wp[ell]); DMA to agin."""
                htp = pp.tile([P, P], F32, tag="htp")
                nc.tensor.transpose(
                    out=htp[:], in_=h[:, j * CH : (j + 1) * CH], identity=ident[:]
                )
                hts = wk.tile([P, P], F32, tag="hts")
                nc.scalar.activation(out=hts[:], in_=htp[:], func=actcopy)
                zp = pp2.tile([P, CH], F32, tag="zp")
                nc.tensor.matmul(
                    out=zp[:], lhsT=hts[:], rhs=Wp[ell][:], start=True, stop=True
                )
                tsl = tloc[:, j * CH : (j + 1) * CH]
                nc.vector.tensor_scalar(
                    out=tsl,
                    in0=zp[:],
                    scalar1=dis_t[:, j : j + 1],
                    scalar2=None,
                    op0=mybir.AluOpType.mult,
                )
                if j < LO_BLOCKS:
                    nc.sync.dma_start(
                        out=agin_lo[ell][j * P : (j + 1) * P, :], in_=tsl
                    )
                if j >= HI_BLK0:
                    r0 = j * P - FLEX0
                    nc.sync.dma_start(
                        out=agin_hi[ell][r0 : r0 + P, :], in_=tsl
                    )

            def emit_ag(ell):
                nc.gpsimd.collective_compute(
                    "AllGather",
                    mybir.AluOpType.bypass,
                    replica_groups=[list(range(NCORES))],
                    ins=[agin_lo[ell][:]],
                    outs=[tab_lo[ell][:]],
                )
                nc.gpsimd.collective_compute(
                    "AllGather",
                    mybir.AluOpType.bypass,
                    replica_groups=[list(range(NCORES))],
                    ins=[agin_hi[ell][:]],
                    outs=[tab_hi[ell][:]],
                )

            # chunk plan per phase: list of (c0, c1) col ranges
            def chunks(total):
                out = []
                c = 0
                while c < total:
                    out.append((c, min(c + CHUNK_COLS, total)))
                    c += CHUNK_COLS
                return out

            lo_chunks = chunks(SLo)
            hi_chunks = chunks(SHi)

            # map global col -> (block, within-block col) per phase
            col_blk_lo = np.searchsorted(clo, np.arange(SLo), side="right") - 1
            col_blk_hi = np.searchsorted(chi, np.arange(SHi), side="right") - 1

            qctr = [0]

            def emit_phase(ell, table, idx_t, w_t, phase_chunks, col_blk, init_done):
                """Gather chunks + STT accumulate columns into accs."""
                for (c0, c1) in phase_chunks:
                    ncols = c1 - c0
                    nidx = ncols * P
                    g_t = gp.tile([P, CHUNK_COLS * CH], BF16, tag="g")
                    nc.gpsimd.dma_gather(
                        g_t[:, : ncols * CH].rearrange("p (s c) -> p s c", c=CH),
                        table[:],
                        idx_t[:, c0 * 8 : c1 * 8],
                        nidx,
                        nidx,
                        CH,
                        queue_num=qctr[0] % NQ,
                        single_packet=False,
                    )
                    qctr[0] += 1
                    for col in range(c0, c1):
                        j = int(col_blk[col])
                        acc_j = accs[:, j * CH : (j + 1) * CH]
                        g_col = g_t[:, (col - c0) * CH : (col - c0 + 1) * CH]
                        w_col = w_t[:, col : col + 1]
                        if not init_done[j]:
                            # first touch: init chain with self-loop + offset
                            emit_self_init(ell, j)
                            init_done[j] = True
                        if is_gps_block(j):
                            tmp = wk.tile([P, CH], F32, tag="gtmp")
                            nc.gpsimd.tensor_scalar(
                                out=tmp[:],
                                in0=g_col,
                                scalar1=w_col,
                                scalar2=None,
                                op0=mybir.AluOpType.mult,
                            )
                            nc.gpsimd.tensor_tensor(
                                out=acc_j, in0=acc_j, in1=tmp[:],
                                op=mybir.AluOpType.add,
                            )
                        else:
                            nc.vector.scalar_tensor_tensor(
                                out=acc_j,
                                in0=g_col,
                                scalar=w_col,
                                in1=acc_j,
                                op0=mybir.AluOpType.mult,
                                op1=mybir.AluOpType.add,
                            )

            def emit_self_init(ell, j):
                """accs[:, j] = tloc_j * dis_j + o_rep[ell]."""
                acc_j = accs[:, j * CH : (j + 1) * CH]
                t_j = tloc[:, j * CH : (j + 1) * CH]
                if is_gps_block(j):
                    tmp = wk.tile([P, CH], F32, tag="gtmp")
                    nc.gpsimd.tensor_scalar(
                        out=tmp[:],
                        in0=t_j,
                        scalar1=dis_t[:, j : j + 1],
                        scalar2=None,
                        op0=mybir.AluOpType.mult,
                    )
                    nc.gpsimd.tensor_tensor(
                        out=acc_j, in0=tmp[:], in1=o_rep[ell][:],
                        op=mybir.AluOpType.add,
                    )
                else:
                    nc.vector.scalar_tensor_tensor(
                        out=acc_j,
                        in0=t_j,
                        scalar=dis_t[:, j : j + 1],
                        in1=o_rep[ell][:],
                        op0=mybir.AluOpType.mult,
                        op1=mybir.AluOpType.add,
                    )

            def emit_epilogue(ell, j):
                """h_j = lrelu(acc_j [+ h_j]); then build next table row."""
                acc_j = accs[:, j * CH : (j + 1) * CH]
                h_j = h[:, j * CH : (j + 1) * CH]
                eng = nc.gpsimd if is_gps_block(j) else nc.vector
                if ell >= 1:
                    eng.tensor_tensor(
                        out=acc_j, in0=acc_j, in1=h_j, op=mybir.AluOpType.add
                    )
                nc.scalar.activation(out=h_j, in_=acc_j, func=lrelu, alpha=0.01)
                if ell < 2:
                    emit_table(ell + 1, j)

            def emit_head(j):
                h_j = h[:, j * CH : (j + 1) * CH]
                htp = pp.tile([P, P], F32, tag="htp")
                nc.tensor.transpose(out=htp[:], in_=h_j, identity=ident[:])
                hts = wk.tile([P, P], F32, tag="hts")
                nc.scalar.activation(out=hts[:], in_=htp[:], func=actcopy)
                z1p = pp2.tile([P, CH], F32, tag="zp")
                nc.tensor.matmul(
                    out=z1p[:], lhsT=hts[:], rhs=lin1_t[:], start=True, stop=True
                )
                z1 = wk.tile([P, CH], F32, tag="z1")
                nc.vector.tensor_tensor(
                    out=z1[:], in0=z1p[:], in1=l1b_t[:], op=mybir.AluOpType.add
                )
                nc.scalar.activation(out=z1[:], in_=z1[:], func=lrelu, alpha=0.01)
                z1tp = pp.tile([P, P], F32, tag="htp")
                nc.tensor.transpose(out=z1tp[:], in_=z1[:], identity=ident[:])
                z1ts = wk.tile([P, P], F32, tag="hts")
                nc.scalar.activation(out=z1ts[:], in_=z1tp[:], func=actcopy)
                z2p = pp2.tile([P, COUT], F32, tag="z2p")
                nc.tensor.matmul(
                    out=z2p[:], lhsT=z1ts[:], rhs=lin2_t[:], start=True, stop=True
                )
                yt = wk.tile([P, COUT], F32, tag="yt")
                nc.vector.tensor_tensor(
                    out=yt[:], in0=z2p[:], in1=l2b_t[:], op=mybir.AluOpType.add
                )
                nc.sync.dma_start(out=y_ext[j * P : (j + 1) * P, :], in_=yt[:])

            # ---- schedule ----
            for j in range(NBLK):
                emit_table(0, j)
            emit_ag(0)
            for ell in range(3):
                init_done = [False] * NBLK
                emit_phase(ell, tab_lo[ell], idx_lo_t, w_lo_t, lo_chunks,
                           col_blk_lo, init_done)
                emit_phase(ell, tab_hi[ell], idx_hi_t, w_hi_t, hi_chunks,
                           col_blk_hi, init_done)
                for j in range(NBLK):
                    if not init_done[j]:
                        emit_self_init(ell, j)
                    emit_epilogue(ell, j)
                    if ell == 2:
                        emit_head(j)
                if ell < 2:
                    emit_ag(ell + 1)

    nc.compile()
    return nc


def kernel(**inputs):
    x = np.asarray(inputs["x"], np.float32)
    edge_index = np.asarray(inputs["edge_index"], np.int64)
    edge_weights = np.asarray(inputs["edge_weights"], np.float32)

    prep = _host_prep(x, edge_index, edge_weights)
    folded = _fold_params(inputs)

    nc = _build_program(prep)

    rep = lambda v: np.tile(np.asarray(v, np.float32)[None, :], (P, 1))
    in_maps = []
    for c in range(NCORES):
        m = {
            "x": prep["x_sh"][c],
            "idx_lo": prep["idx_lo"][c],
            "idx_hi": prep["idx_hi"][c],
            "w_lo": prep["w_lo"][c],
            "w_hi": prep["w_hi"][c],
            "dis": prep["dis_sh"][c],
            "lin1": np.asarray(inputs["lin1_w"], np.float32),
            "lin2": np.asarray(inputs["lin2_w"], np.float32),
            "l1b": rep(inputs["lin1_b"]),
            "l2b": rep(inputs["lin2_b"]),
        }
        for ell in range(3):
            m[f"Wp{ell}"] = folded[f"Wp{ell}"]
            m[f"o{ell}"] = rep(folded[f"o{ell}"])
        in_maps.append(m)

    res = run_bass_kernel_spmd(nc, in_maps, core_ids=list(range(NCORES)))
    global _last_results
    _last_results = res

    out = np.empty((N, COUT), np.float32)
    order = prep["order"]
    for c in range(NCORES):
        ranks = np.arange(c, N, NCORES)
        out[order[ranks]] = res.results[c]["y"][: len(ranks)]
    return out


# revision 14
# speedup vs baseline: 1.9109x; 1.9109x over previous
"""GCN message-passing kernel for 8 trn2 NeuronCores (bass/Tile).

Sharding: nodes are degree-sorted and dealt round-robin across 8 cores
(graph-parallel, dst-sharded).  Each core computes t = dis*(h @ W') for
its node shard; two AllGathers replicate the scaled table in bf16 (a lo
half and an overlapping hi half so every gather index fits in int16);
each core fetches its in-edge source rows with bulk SWDGE dma_gather
instructions (4 queues, GpSimd issues nothing else) and reduces them
per destination block either on the Vector engine (wide broadcast
multiply + halving-tree adds) or on the Tensor engine (per-column
diagonal matmuls accumulating in PSUM).  BatchNorm (eval mode) is
folded into the weights/offsets on the host; self-loops are applied
from the local table copy without any gather.
"""

import sys

sys.path.insert(0, "/opt/trn_rl_repo")

import numpy as np
import ml_dtypes

import concourse.bass as bass
import concourse.bacc as bacc
import concourse.mybir as mybir
from concourse.bass_utils import run_bass_kernel_spmd
from concourse.masks import make_identity
from concourse.tile import TileContext

N = 50000
E = 800000
CIN = 128
CH = 128
COUT = 64
EPS = 1e-5
NCORES = 8
P = 128
SHARD = 6272          # 49 blocks * 128
NBLK = SHARD // P     # 49
AGL = 4096            # lo AllGather rows per core  (locals [0, 4096))
FLEX0 = 2176          # hi AllGather covers locals [2176, 6272)
AGH = 4096
LO_BLOCKS = AGL // P      # 32
HI_BLK0 = FLEX0 // P      # 17
MAX_CHUNK = 44            # max gather cols per chunk (SP=0, 4 queues)
NQ = 4

F32 = mybir.dt.float32
BF16 = mybir.dt.bfloat16
I16 = mybir.dt.int16


def _is_tensor_block(j):
    """Blocks reduced on TensorE (diag-matmul PSUM path) vs Vector."""
    return (j % 7) < 3


def _host_prep(x, edge_index, edge_weights):
    """Shard nodes, build the per-core gather/slot layout (pure numpy)."""
    src = edge_index[0].astype(np.int64)
    dst = edge_index[1].astype(np.int64)
    ew = edge_weights.astype(np.float64)

    degc = np.bincount(dst, minlength=N)
    wdeg = np.bincount(dst, weights=ew, minlength=N) + 1.0
    dis = (1.0 / np.sqrt(wdeg)).astype(np.float32)

    order = np.argsort(degc, kind="stable")
    rank = np.empty(N, np.int64)
    rank[order] = np.arange(N)
    core_of = rank % NCORES
    local_of = rank // NCORES

    lo_row = core_of * AGL + local_of                 # valid iff local < AGL
    hi_row = core_of * AGH + (local_of - FLEX0)       # valid iff local >= FLEX0

    s_local = local_of[src]
    cat = np.where(s_local < FLEX0, 0, np.where(s_local < AGL, 1, 2))

    d_core = core_of[dst]
    d_local = local_of[dst]
    d_blk = d_local // P

    gl_node = rank[dst]
    a_cnt = np.bincount(gl_node[cat == 0], minlength=N)
    f_cnt = np.bincount(gl_node[cat == 1], minlength=N)
    b_cnt = np.bincount(gl_node[cat == 2], minlength=N)
    d_cnt = a_cnt + f_cnt + b_cnt
    pad_to = NBLK * P * NCORES  # 50176 > N: pad counts with zeros

    def _blkmax(cnt):
        c = np.zeros(pad_to, cnt.dtype)
        c[:N] = cnt
        return c.reshape(NBLK, P * NCORES).max(axis=1)

    A_j = _blkmax(a_cnt)
    B_j = _blkmax(b_cnt)
    D_j = _blkmax(d_cnt)
    S_j = np.maximum(D_j, A_j + B_j)
    Whi = np.maximum(B_j, 1)
    Wlo = np.maximum(S_j - Whi, 1)

    clo = np.concatenate([[0], np.cumsum(Wlo)]).astype(np.int64)
    chi = np.concatenate([[0], np.cumsum(Whi)]).astype(np.int64)
    SLo, SHi = int(clo[-1]), int(chi[-1])

    blk_of_rank = np.minimum(np.arange(N) // (P * NCORES), NBLK - 1)
    lo_quota = np.maximum(a_cnt, d_cnt - Whi[blk_of_rank])

    eidx = np.arange(E)
    fe = eidx[cat == 1]
    fe_sorted = fe[np.argsort(gl_node[fe], kind="stable")]
    gn_f = gl_node[fe_sorted]
    starts = np.searchsorted(gn_f, np.arange(N))
    rank_in_node = np.arange(len(fe_sorted)) - starts[gn_f]
    goes_lo = rank_in_node < (lo_quota[gn_f] - a_cnt[gn_f])
    is_lo = np.zeros(E, bool)
    is_lo[cat == 0] = True
    is_lo[fe_sorted[goes_lo]] = True

    def slots_for(mask):
        ee = eidx[mask]
        ee = ee[np.argsort(gl_node[ee], kind="stable")]
        gn = gl_node[ee]
        st = np.searchsorted(gn, np.arange(N))
        sl = np.arange(len(ee)) - st[gn]
        return ee, sl

    ee_lo, sl_lo = slots_for(is_lo)
    ee_hi, sl_hi = slots_for(~is_lo)

    idx_lo = np.zeros((NCORES, SLo * P), np.int16)
    idx_hi = np.zeros((NCORES, SHi * P), np.int16)
    w_lo = np.zeros((NCORES, P, SLo), np.float32)
    w_hi = np.zeros((NCORES, P, SHi), np.float32)

    for arr_i, arr_w, ee, sl, row_of, cbase in (
        (idx_lo, w_lo, ee_lo, sl_lo, lo_row, clo),
        (idx_hi, w_hi, ee_hi, sl_hi, hi_row, chi),
    ):
        c = d_core[ee]
        p = d_local[ee] % P
        col = cbase[d_blk[ee]] + sl
        pos = col * P + p
        arr_i[c, pos] = row_of[src[ee]].astype(np.int16)
        arr_w[c, p, col] = (ew[ee] * dis[dst[ee]]).astype(np.float32)

    def wrap_idx(a):
        w16 = a.reshape(-1, 16).T.copy()
        return np.ascontiguousarray(np.tile(w16, (8, 1)))

    idx_lo_t = np.stack([wrap_idx(idx_lo[c]) for c in range(NCORES)])
    idx_hi_t = np.stack([wrap_idx(idx_hi[c]) for c in range(NCORES)])

    x_sh = np.zeros((NCORES, SHARD, CIN), np.float32)
    dis_sh = np.ones((NCORES, P, NBLK), np.float32)
    for c in range(NCORES):
        ranks = np.arange(c, N, NCORES)
        loc = ranks // NCORES
        x_sh[c, loc] = x[order[ranks]]
        dflat = np.ones(SHARD, np.float32)
        dflat[loc] = dis[order[ranks]]
        dis_sh[c] = dflat.reshape(NBLK, P).T

    return dict(
        order=order,
        dis=dis,
        Wlo=Wlo.astype(int),
        Whi=Whi.astype(int),
        clo=clo,
        chi=chi,
        SLo=SLo,
        SHi=SHi,
        idx_lo=idx_lo_t,
        idx_hi=idx_hi_t,
        w_lo=w_lo,
        w_hi=w_hi,
        x_sh=x_sh,
        dis_sh=dis_sh,
        raw_idx_lo=idx_lo,
        raw_idx_hi=idx_hi,
    )


def _fold_params(inputs):
    out = {}
    for ell in range(3):
        if ell == 0:
            W = np.asarray(inputs["w1"], np.float32)
            cb = np.asarray(inputs["b1"], np.float32)
            g = np.asarray(inputs["bn1_g"], np.float32)
            b = np.asarray(inputs["bn1_b"], np.float32)
            m = np.asarray(inputs["bn1_m"], np.float32)
            v = np.asarray(inputs["bn1_v"], np.float32)
        else:
            W = np.asarray(inputs["conv_ws"], np.float32)[ell - 1]
            cb = np.asarray(inputs["conv_bs"], np.float32)[ell - 1]
            g = np.asarray(inputs["bns_g"], np.float32)[ell - 1]
            b = np.asarray(inputs["bns_b"], np.float32)[ell - 1]
            m = np.asarray(inputs["bns_m"], np.float32)[ell - 1]
            v = np.asarray(inputs["bns_v"], np.float32)[ell - 1]
        s_g = g / np.sqrt(v + EPS)
        out[f"Wp{ell}"] = np.ascontiguousarray(W * s_g[None, :])
        out[f"o{ell}"] = (cb - m) * s_g + b
    return out


def _plan(prep):
    """Chunk plan: block-aligned col groups per phase, plus diag-A layout.

    Returns list of chunks: (phase, c0, c1, blocks, tc0) where blocks is
    [(j, b0, b1)] col subranges per block and tc0 the chunk's offset into
    the diag-A array; plus the total tensor-block col count."""
    chunks = []
    tcols = 0
    for phase, (Wp_, cbase, S) in enumerate(
        ((prep["Wlo"], prep["clo"], prep["SLo"]),
         (prep["Whi"], prep["chi"], prep["SHi"]))
    ):
        j = 0
        while j < NBLK:
            c0 = int(cbase[j])
            jend = j
            while jend < NBLK and int(cbase[jend + 1]) - c0 <= MAX_CHUNK:
                jend += 1
            if jend == j:  # single block wider than MAX_CHUNK: split it
                b0 = c0
                while b0 < int(cbase[j + 1]):
                    b1 = min(b0 + MAX_CHUNK, int(cbase[j + 1]))
                    tc0 = tcols
                    if _is_tensor_block(j):
                        tcols += b1 - b0
                    chunks.append((phase, b0, b1, [(j, b0, b1)], tc0))
                    b0 = b1
                j += 1
                continue
            c1 = int(cbase[jend])
            blocks = []
            tc0 = tcols
            for jj in range(j, jend):
                bb0, bb1 = int(cbase[jj]), int(cbase[jj + 1])
                if bb1 > bb0:
                    blocks.append((jj, bb0, bb1))
                    if _is_tensor_block(jj):
                        tcols += bb1 - bb0
            chunks.append((phase, c0, c1, blocks, tc0))
            j = jend
    return chunks, tcols


def _build_diag_a(prep, chunks, tcols):
    """Per-core diag-A array [P, tcols*P] bf16 in chunk order."""
    a = np.zeros((NCORES, P, tcols, P), ml_dtypes.bfloat16)
    rng_p = np.arange(P)
    for c in range(NCORES):
        for (phase, c0, c1, blocks, tc0) in chunks:
            w = prep["w_lo"][c] if phase == 0 else prep["w_hi"][c]
            t = tc0
            for (j, b0, b1) in blocks:
                if not _is_tensor_block(j):
                    continue
                ncol = b1 - b0
                # a[c, p, t+k, p] = w[p, b0+k]
                a[c, rng_p[:, None], t + np.arange(ncol)[None, :], rng_p[:, None]] = (
                    w[:, b0:b1].astype(ml_dtypes.bfloat16)
                )
                t += ncol
    return a.reshape(NCORES, P, tcols * P)


def _build_program(prep, chunks, tcols):
    clo, chi = prep["clo"], prep["chi"]
    SLo, SHi = prep["SLo"], prep["SHi"]

    nc = bacc.Bacc(num_swdge_queues=NQ)

    x_ext = nc.declare_dram_parameter("x", [SHARD, CIN], F32, isOutput=False)
    ilo_ext = nc.declare_dram_parameter("idx_lo", [P, SLo * 8], I16, isOutput=False)
    ihi_ext = nc.declare_dram_parameter("idx_hi", [P, SHi * 8], I16, isOutput=False)
    wlo_ext = nc.declare_dram_parameter("w_lo", [P, SLo], F32, isOutput=False)
    whi_ext = nc.declare_dram_parameter("w_hi", [P, SHi], F32, isOutput=False)
    da_ext = nc.declare_dram_parameter("diag_a", [P, max(tcols, 1) * P], BF16,
                                       isOutput=False)
    dis_ext = nc.declare_dram_parameter("dis", [P, NBLK], F32, isOutput=False)
    wp_ext = [
        nc.declare_dram_parameter(f"Wp{ell}", [CH, CH], F32, isOutput=False)
        for ell in range(3)
    ]
    o_ext = [
        nc.declare_dram_parameter(f"o{ell}", [P, CH], F32, isOutput=False)
        for ell in range(3)
    ]
    lin1_ext = nc.declare_dram_parameter("lin1", [CH, CH], F32, isOutput=False)
    lin2_ext = nc.declare_dram_parameter("lin2", [CH, COUT], F32, isOutput=False)
    l1b_ext = nc.declare_dram_parameter("l1b", [P, CH], F32, isOutput=False)
    l2b_ext = nc.declare_dram_parameter("l2b", [P, COUT], F32, isOutput=False)
    y_ext = nc.declare_dram_parameter("y", [SHARD, COUT], F32, isOutput=True)

    lrelu = mybir.ActivationFunctionType.Lrelu
    actcopy = mybir.ActivationFunctionType.Copy
    ADD = mybir.AluOpType.add
    MULT = mybir.AluOpType.mult

    with TileContext(nc) as tc:
        with (
            tc.tile_pool(name="const", bufs=1) as constp,
            tc.tile_pool(name="gpool", bufs=3) as gp,
            tc.tile_pool(name="apool", bufs=3) as ap_pool,
            tc.tile_pool(name="work", bufs=4) as wk,
            tc.tile_pool(name="psum", bufs=2, space="PSUM") as pp,
            tc.tile_pool(name="psum2", bufs=2, space="PSUM") as pp2,
            tc.tile_pool(name="pacc", bufs=3, space="PSUM") as pacc,
            tc.tile_pool(name="dram", bufs=1, space="DRAM") as dp,
        ):
            ident = constp.tile([P, P], F32)
            make_identity(nc, ident[:])
            idx_lo_t = constp.tile([P, SLo * 8], I16)
            nc.sync.dma_start(out=idx_lo_t[:], in_=ilo_ext[:])
            idx_hi_t = constp.tile([P, SHi * 8], I16)
            nc.sync.dma_start(out=idx_hi_t[:], in_=ihi_ext[:])
            w_lo_t = constp.tile([P, SLo], F32)
            nc.sync.dma_start(out=w_lo_t[:], in_=wlo_ext[:])
            w_hi_t = constp.tile([P, SHi], F32)
            nc.sync.dma_start(out=w_hi_t[:], in_=whi_ext[:])
            dis_t = constp.tile([P, NBLK], F32)
            nc.sync.dma_start(out=dis_t[:], in_=dis_ext[:])
            Wp, o_rep = [], []
            for ell in range(3):
                t = constp.tile([P, CH], F32, name=f"Wp{ell}")
                nc.sync.dma_start(out=t[:], in_=wp_ext[ell][:])
                Wp.append(t)
                t2 = constp.tile([P, CH], F32, name=f"o{ell}")
                nc.sync.dma_start(out=t2[:], in_=o_ext[ell][:])
                o_rep.append(t2)
            lin1_t = constp.tile([P, CH], F32)
            nc.sync.dma_start(out=lin1_t[:], in_=lin1_ext[:])
            lin2_t = constp.tile([P, COUT], F32)
            nc.sync.dma_start(out=lin2_t[:], in_=lin2_ext[:])
            l1b_t = constp.tile([P, CH], F32)
            nc.sync.dma_start(out=l1b_t[:], in_=l1b_ext[:])
            l2b_t = constp.tile([P, COUT], F32)
            nc.sync.dma_start(out=l2b_t[:], in_=l2b_ext[:])

            h = constp.tile([P, NBLK * CH], F32)
            for j in range(NBLK):
                nc.sync.dma_start(
                    out=h[:, j * CH : (j + 1) * CH],
                    in_=x_ext[j * P : (j + 1) * P, :],
                )
            accs = constp.tile([P, NBLK * CH], F32)
            tloc = constp.tile([P, NBLK * CH], BF16)

            tb_list = [j for j in range(NBLK) if _is_tensor_block(j)]

            agin_lo, agin_hi, tab_lo, tab_hi = [], [], [], []
            for ell in range(3):
                agin_lo.append(dp.tile([AGL, CH], BF16, name=f"aglo{ell}"))
                agin_hi.append(dp.tile([AGH, CH], BF16, name=f"aghi{ell}"))
                tab_lo.append(
                    dp.tile([NCORES * AGL, CH], BF16, name=f"tlo{ell}",
                            addr_space="Shared")
                )
                tab_hi.append(
                    dp.tile([NCORES * AGH, CH], BF16, name=f"thi{ell}",
                            addr_space="Shared")
                )

            def emit_table(ell, j):
                """tloc[:, j] = dis * (h_j @ Wp[ell]); DMA to agin."""
                htp = pp.tile([P, P], F32, tag="htp")
                nc.tensor.transpose(
                    out=htp[:], in_=h[:, j * CH : (j + 1) * CH], identity=ident[:]
                )
                hts = wk.tile([P, P], F32, tag="hts")
                nc.scalar.activation(out=hts[:], in_=htp[:], func=actcopy)
                zp = pp2.tile([P, CH], F32, tag="zp")
                nc.tensor.matmul(
                    out=zp[:], lhsT=hts[:], rhs=Wp[ell][:], start=True, stop=True
                )
                tsl = tloc[:, j * CH : (j + 1) * CH]
                nc.scalar.activation(
                    out=tsl, in_=zp[:], func=actcopy, scale=dis_t[:, j : j + 1]
                )
                if j < LO_BLOCKS:
                    nc.sync.dma_start(
                        out=agin_lo[ell][j * P : (j + 1) * P, :], in_=tsl
                    )
                if j >= HI_BLK0:
                    r0 = j * P - FLEX0
                    nc.sync.dma_start(out=agin_hi[ell][r0 : r0 + P, :], in_=tsl)

            def emit_ag(ell):
                nc.gpsimd.collective_compute(
                    "AllGather",
                    mybir.AluOpType.bypass,
                    replica_groups=[list(range(NCORES))],
                    ins=[agin_lo[ell][:]],
                    outs=[tab_lo[ell][:]],
                )
                nc.gpsimd.collective_compute(
                    "AllGather",
                    mybir.AluOpType.bypass,
                    replica_groups=[list(range(NCORES))],
                    ins=[agin_hi[ell][:]],
                    outs=[tab_hi[ell][:]],
                )

            qctr = [0]
            # per-(phase, block) matmul column totals, for start/stop flags
            tb_total = {}
            for (phase, c0, c1, blocks, tc0) in chunks:
                for (j, b0, b1) in blocks:
                    if _is_tensor_block(j):
                        tb_total[(phase, j)] = tb_total.get((phase, j), 0) + (b1 - b0)

            def emit_layer(ell):
                slab_state = {}   # j -> True once acc_j initialized
                tb_chain = {}     # (phase, j) -> [psum_tile, done]

                def merge_slab(j, src_ap):
                    """acc_j (+)= src_ap, with fused self+offset on first."""
                    acc_j = accs[:, j * CH : (j + 1) * CH]
                    if j not in slab_state:
                        nc.vector.scalar_tensor_tensor(
                            out=acc_j,
                            in0=tloc[:, j * CH : (j + 1) * CH],
                            scalar=dis_t[:, j : j + 1],
                            in1=src_ap,
                            op0=MULT,
                            op1=ADD,
                        )
                        nc.vector.tensor_tensor(
                            out=acc_j, in0=acc_j, in1=o_rep[ell][:], op=ADD
                        )
                        slab_state[j] = True
                    else:
                        nc.vector.tensor_tensor(
                            out=acc_j, in0=acc_j, in1=src_ap, op=ADD
                        )

                for (phase, c0, c1, blocks, tc0) in chunks:
                    table = tab_lo[ell] if phase == 0 else tab_hi[ell]
                    idx_t = idx_lo_t if phase == 0 else idx_hi_t
                    w_t = w_lo_t if phase == 0 else w_hi_t
                    ncols = c1 - c0
                    nidx = ncols * P
                    g_t = gp.tile([P, MAX_CHUNK * CH], BF16, tag="g")
                    nc.gpsimd.dma_gather(
                        g_t[:, : ncols * CH].rearrange("p (s c) -> p s c", c=CH),
                        table[:],
                        idx_t[:, c0 * 8 : c1 * 8],
                        nidx,
                        nidx,
                        CH,
                        queue_num=qctr[0] % NQ,
                        single_packet=False,
                    )
                    qctr[0] += 1
                    # how many tensor cols in this chunk?
                    t_ncols = sum(
                        b1 - b0 for (j, b0, b1) in blocks if _is_tensor_block(j)
                    )
                    if t_ncols:
                        a_t = ap_pool.tile([P, MAX_CHUNK * P], BF16, tag="a")
                        nc.sync.dma_start(
                            out=a_t[:, : t_ncols * P],
                            in_=da_ext[:, tc0 * P : (tc0 + t_ncols) * P],
                        )
                    t_off = 0
                    for (j, b0, b1) in blocks:
                        W = b1 - b0
                        goff = (b0 - c0) * CH
                        if _is_tensor_block(j):
                            key = (phase, j)
                            if key not in tb_chain:
                                tb_chain[key] = [pacc.tile([P, CH], F32, tag="pa", name="pa"), 0]
                            pt, done = tb_chain[key]
                            total = tb_total[key]
                            for k in range(W):
                                nc.tensor.matmul(
                                    out=pt[:],
                                    lhsT=a_t[:, (t_off + k) * P : (t_off + k + 1) * P],
                                    rhs=g_t[:, goff + k * CH : goff + (k + 1) * CH],
                                    start=(done == 0),
                                    stop=(done == total - 1),
                                )
                                done += 1
                            tb_chain[key][1] = done
                            if done == total:
                                merge_slab(j, pt[:])
                            t_off += W
                        else:
                            sl = g_t[:, goff : goff + W * CH]
                            wb = (
                                w_t[:, b0:b1]
                                .unsqueeze(2)
                                .broadcast_to([P, W, CH])
                            )
                            nc.vector.tensor_tensor(out=sl, in0=sl, in1=wb, op=MULT)
                            n = W
                            while n > 1:
                                if n % 2 == 1:
                                    nc.vector.tensor_tensor(
                                        out=sl[:, 0:CH],
                                        in0=sl[:, 0:CH],
                                        in1=sl[:, (n - 1) * CH : n * CH],
                                        op=ADD,
                                    )
                                    n -= 1
                                hh = n // 2
                                nc.vector.tensor_tensor(
                                    out=sl[:, : hh * CH],
                                    in0=sl[:, : hh * CH],
                                    in1=sl[:, hh * CH : 2 * hh * CH],
                                    op=ADD,
                                )
                                n = hh
                            merge_slab(j, sl[:, 0:CH])

                # epilogues
                for j in range(NBLK):
                    acc_j = accs[:, j * CH : (j + 1) * CH]
                    h_j = h[:, j * CH : (j + 1) * CH]
                    if j not in slab_state:
                        # no edge slots at all: acc = tloc*dis + o
                        nc.vector.scalar_tensor_tensor(
                            out=acc_j,
                            in0=tloc[:, j * CH : (j + 1) * CH],
                            scalar=dis_t[:, j : j + 1],
                            in1=o_rep[ell][:],
                            op0=MULT,
                            op1=ADD,
                        )
                    if ell >= 1:
                        nc.vector.tensor_tensor(
                            out=acc_j, in0=acc_j, in1=h_j, op=ADD
                        )
                    nc.scalar.activation(out=h_j, in_=acc_j, func=lrelu, alpha=0.01)
                    if ell < 2:
                        emit_table(ell + 1, j)
                    else:
                        emit_head(j)

            def emit_head(j):
                h_j = h[:, j * CH : (j + 1) * CH]
                htp = pp.tile([P, P], F32, tag="htp")
                nc.tensor.transpose(out=htp[:], in_=h_j, identity=ident[:])
                hts = wk.tile([P, P], F32, tag="hts")
                nc.scalar.activation(out=hts[:], in_=htp[:], func=actcopy)
                z1p = pp2.tile([P, CH], F32, tag="zp")
                nc.tensor.matmul(
                    out=z1p[:], lhsT=hts[:], rhs=lin1_t[:], start=True, stop=True
                )
                z1 = wk.tile([P, CH], F32, tag="z1")
                nc.vector.tensor_tensor(out=z1[:], in0=z1p[:], in1=l1b_t[:], op=ADD)
                nc.scalar.activation(out=z1[:], in_=z1[:], func=lrelu, alpha=0.01)
                z1tp = pp.tile([P, P], F32, tag="htp")
                nc.tensor.transpose(out=z1tp[:], in_=z1[:], identity=ident[:])
                z1ts = wk.tile([P, P], F32, tag="hts")
                nc.scalar.activation(out=z1ts[:], in_=z1tp[:], func=actcopy)
                z2p = pp2.tile([P, CH], F32, tag="zp")
                nc.tensor.matmul(
                    out=z2p[:, :COUT], lhsT=z1ts[:], rhs=lin2_t[:], start=True,
                    stop=True,
                )
                yt = wk.tile([P, COUT], F32, tag="yt")
                nc.vector.tensor_tensor(
                    out=yt[:], in0=z2p[:, :COUT], in1=l2b_t[:], op=ADD
                )
                nc.sync.dma_start(out=y_ext[j * P : (j + 1) * P, :], in_=yt[:])

            # ---- schedule ----
            for j in range(NBLK):
                emit_table(0, j)
            emit_ag(0)
            for ell in range(3):
                emit_layer(ell)
                if ell < 2:
                    emit_ag(ell + 1)

    nc.compile()
    return nc


def kernel(**inputs):
    x = np.asarray(inputs["x"], np.float32)
    edge_index = np.asarray(inputs["edge_index"], np.int64)
    edge_weights = np.asarray(inputs["edge_weights"], np.float32)

    prep = _host_prep(x, edge_index, edge_weights)
    folded = _fold_params(inputs)
    chunks, tcols = _plan(prep)
    diag_a = _build_diag_a(prep, chunks, tcols)

    nc = _build_program(prep, chunks, tcols)

    rep = lambda v: np.tile(np.asarray(v, np.float32)[None, :], (P, 1))
    in_maps = []
    for c in range(NCORES):
        m = {
            "x": prep["x_sh"][c],
            "idx_lo": prep["idx_lo"][c],
            "idx_hi": prep["idx_hi"][c],
            "w_lo": prep["w_lo"][c],
            "w_hi": prep["w_hi"][c],
            "diag_a": diag_a[c] if tcols else np.zeros((P, P), ml_dtypes.bfloat16),
            "dis": prep["dis_sh"][c],
            "lin1": np.asarray(inputs["lin1_w"], np.float32),
            "lin2": np.asarray(inputs["lin2_w"], np.float32),
            "l1b": rep(inputs["lin1_b"]),
            "l2b": rep(inputs["lin2_b"]),
        }
        for ell in range(3):
            m[f"Wp{ell}"] = folded[f"Wp{ell}"]
            m[f"o{ell}"] = rep(folded[f"o{ell}"])
        in_maps.append(m)

    res = run_bass_kernel_spmd(nc, in_maps, core_ids=list(range(NCORES)))
    global _last_results
    _last_results = res

    out = np.empty((N, COUT), np.float32)
    order = prep["order"]
    for c in range(NCORES):
        ranks = np.arange(c, N, NCORES)
        out[order[ranks]] = res.results[c]["y"][: len(ranks)]
    return out


# revision 16
# speedup vs baseline: 2.0159x; 1.0550x over previous
"""GCN message-passing kernel for 8 trn2 NeuronCores (bass/Tile).

Sharding: nodes are degree-sorted and dealt round-robin across 8 cores
(graph-parallel, dst-sharded).  Each core computes t = dis*(h @ W') for
its node shard; two AllGathers replicate the scaled table in bf16 (a lo
half and an overlapping hi half so every gather index fits in int16);
each core fetches its in-edge source rows with bulk SWDGE dma_gather
instructions (4 queues, GpSimd issues nothing else) and reduces them
per destination block either on the Vector engine (wide broadcast
multiply + halving-tree adds) or on the Tensor engine (per-column
diagonal matmuls accumulating in PSUM).  BatchNorm (eval mode) is
folded into the weights/offsets on the host; self-loops are applied
from the local table copy without any gather.
"""

import sys

sys.path.insert(0, "/opt/trn_rl_repo")

import numpy as np
import ml_dtypes

import concourse.bass as bass
import concourse.bacc as bacc
import concourse.mybir as mybir
from concourse.bass_utils import run_bass_kernel_spmd
from concourse.masks import make_identity
from concourse.tile import TileContext

N = 50000
E = 800000
CIN = 128
CH = 128
COUT = 64
EPS = 1e-5
NCORES = 8
P = 128
SHARD = 6272          # 49 blocks * 128
NBLK = SHARD // P     # 49
AGL = 4096            # lo AllGather rows per core  (locals [0, 4096))
FLEX0 = 2176          # hi AllGather covers locals [2176, 6272)
AGH = 4096
LO_BLOCKS = AGL // P      # 32
HI_BLK0 = FLEX0 // P      # 17
MAX_CHUNK = 42            # max gather cols per chunk tile
SUB_COLS = 7              # cols per sub-gather (896 descs < 1024 ring)
NQ = 4

F32 = mybir.dt.float32
BF16 = mybir.dt.bfloat16
I16 = mybir.dt.int16


def _is_tensor_block(j):
    """Blocks reduced on TensorE (diag-matmul PSUM path) vs Vector."""
    return (j % 7) < 3


def _host_prep(x, edge_index, edge_weights):
    """Shard nodes, build the per-core gather/slot layout (pure numpy)."""
    src = edge_index[0].astype(np.int64)
    dst = edge_index[1].astype(np.int64)
    ew = edge_weights.astype(np.float64)

    degc = np.bincount(dst, minlength=N)
    wdeg = np.bincount(dst, weights=ew, minlength=N) + 1.0
    dis = (1.0 / np.sqrt(wdeg)).astype(np.float32)

    order = np.argsort(degc, kind="stable")
    rank = np.empty(N, np.int64)
    rank[order] = np.arange(N)
    core_of = rank % NCORES
    local_of = rank // NCORES

    lo_row = core_of * AGL + local_of                 # valid iff local < AGL
    hi_row = core_of * AGH + (local_of - FLEX0)       # valid iff local >= FLEX0

    s_local = local_of[src]
    cat = np.where(s_local < FLEX0, 0, np.where(s_local < AGL, 1, 2))

    d_core = core_of[dst]
    d_local = local_of[dst]
    d_blk = d_local // P

    gl_node = rank[dst]
    a_cnt = np.bincount(gl_node[cat == 0], minlength=N)
    f_cnt = np.bincount(gl_node[cat == 1], minlength=N)
    b_cnt = np.bincount(gl_node[cat == 2], minlength=N)
    d_cnt = a_cnt + f_cnt + b_cnt
    pad_to = NBLK * P * NCORES  # 50176 > N: pad counts with zeros

    def _blkmax(cnt):
        c = np.zeros(pad_to, cnt.dtype)
        c[:N] = cnt
        return c.reshape(NBLK, P * NCORES).max(axis=1)

    A_j = _blkmax(a_cnt)
    B_j = _blkmax(b_cnt)
    D_j = _blkmax(d_cnt)
    S_j = np.maximum(D_j, A_j + B_j)
    Whi = np.maximum(B_j, 1)
    Wlo = np.maximum(S_j - Whi, 1)

    clo = np.concatenate([[0], np.cumsum(Wlo)]).astype(np.int64)
    chi = np.concatenate([[0], np.cumsum(Whi)]).astype(np.int64)
    SLo, SHi = int(clo[-1]), int(chi[-1])

    blk_of_rank = np.minimum(np.arange(N) // (P * NCORES), NBLK - 1)
    lo_quota = np.maximum(a_cnt, d_cnt - Whi[blk_of_rank])

    eidx = np.arange(E)
    fe = eidx[cat == 1]
    fe_sorted = fe[np.argsort(gl_node[fe], kind="stable")]
    gn_f = gl_node[fe_sorted]
    starts = np.searchsorted(gn_f, np.arange(N))
    rank_in_node = np.arange(len(fe_sorted)) - starts[gn_f]
    goes_lo = rank_in_node < (lo_quota[gn_f] - a_cnt[gn_f])
    is_lo = np.zeros(E, bool)
    is_lo[cat == 0] = True
    is_lo[fe_sorted[goes_lo]] = True

    def slots_for(mask):
        ee = eidx[mask]
        ee = ee[np.argsort(gl_node[ee], kind="stable")]
        gn = gl_node[ee]
        st = np.searchsorted(gn, np.arange(N))
        sl = np.arange(len(ee)) - st[gn]
        return ee, sl

    ee_lo, sl_lo = slots_for(is_lo)
    ee_hi, sl_hi = slots_for(~is_lo)

    idx_lo = np.zeros((NCORES, SLo * P), np.int16)
    idx_hi = np.zeros((NCORES, SHi * P), np.int16)
    w_lo = np.zeros((NCORES, P, SLo), np.float32)
    w_hi = np.zeros((NCORES, P, SHi), np.float32)

    for arr_i, arr_w, ee, sl, row_of, cbase in (
        (idx_lo, w_lo, ee_lo, sl_lo, lo_row, clo),
        (idx_hi, w_hi, ee_hi, sl_hi, hi_row, chi),
    ):
        c = d_core[ee]
        p = d_local[ee] % P
        col = cbase[d_blk[ee]] + sl
        pos = col * P + p
        arr_i[c, pos] = row_of[src[ee]].astype(np.int16)
        arr_w[c, p, col] = (ew[ee] * dis[dst[ee]]).astype(np.float32)

    def wrap_idx(a):
        w16 = a.reshape(-1, 16).T.copy()
        return np.ascontiguousarray(np.tile(w16, (8, 1)))

    idx_lo_t = np.stack([wrap_idx(idx_lo[c]) for c in range(NCORES)])
    idx_hi_t = np.stack([wrap_idx(idx_hi[c]) for c in range(NCORES)])

    x_sh = np.zeros((NCORES, SHARD, CIN), np.float32)
    dis_sh = np.ones((NCORES, P, NBLK), np.float32)
    for c in range(NCORES):
        ranks = np.arange(c, N, NCORES)
        loc = ranks // NCORES
        x_sh[c, loc] = x[order[ranks]]
        dflat = np.ones(SHARD, np.float32)
        dflat[loc] = dis[order[ranks]]
        dis_sh[c] = dflat.reshape(NBLK, P).T

    return dict(
        order=order,
        dis=dis,
        Wlo=Wlo.astype(int),
        Whi=Whi.astype(int),
        clo=clo,
        chi=chi,
        SLo=SLo,
        SHi=SHi,
        idx_lo=idx_lo_t,
        idx_hi=idx_hi_t,
        w_lo=w_lo,
        w_hi=w_hi,
        x_sh=x_sh,
        dis_sh=dis_sh,
        raw_idx_lo=idx_lo,
        raw_idx_hi=idx_hi,
    )


def _fold_params(inputs):
    out = {}
    for ell in range(3):
        if ell == 0:
            W = np.asarray(inputs["w1"], np.float32)
            cb = np.asarray(inputs["b1"], np.float32)
            g = np.asarray(inputs["bn1_g"], np.float32)
            b = np.asarray(inputs["bn1_b"], np.float32)
            m = np.asarray(inputs["bn1_m"], np.float32)
            v = np.asarray(inputs["bn1_v"], np.float32)
        else:
            W = np.asarray(inputs["conv_ws"], np.float32)[ell - 1]
            cb = np.asarray(inputs["conv_bs"], np.float32)[ell - 1]
            g = np.asarray(inputs["bns_g"], np.float32)[ell - 1]
            b = np.asarray(inputs["bns_b"], np.float32)[ell - 1]
            m = np.asarray(inputs["bns_m"], np.float32)[ell - 1]
            v = np.asarray(inputs["bns_v"], np.float32)[ell - 1]
        s_g = g / np.sqrt(v + EPS)
        out[f"Wp{ell}"] = np.ascontiguousarray(W * s_g[None, :])
        out[f"o{ell}"] = (cb - m) * s_g + b
    return out


def _plan(prep):
    """Chunk plan: block-aligned col groups per phase, plus diag-A layout.

    Returns list of chunks: (phase, c0, c1, blocks, tc0) where blocks is
    [(j, b0, b1)] col subranges per block and tc0 the chunk's offset into
    the diag-A array; plus the total tensor-block col count."""
    chunks = []
    tcols = 0
    for phase, (Wp_, cbase, S) in enumerate(
        ((prep["Wlo"], prep["clo"], prep["SLo"]),
         (prep["Whi"], prep["chi"], prep["SHi"]))
    ):
        j = 0
        while j < NBLK:
            c0 = int(cbase[j])
            jend = j
            while jend < NBLK and int(cbase[jend + 1]) - c0 <= MAX_CHUNK:
                jend += 1
            if jend == j:  # single block wider than MAX_CHUNK: split it
                b0 = c0
                while b0 < int(cbase[j + 1]):
                    b1 = min(b0 + MAX_CHUNK, int(cbase[j + 1]))
                    tc0 = tcols
                    if _is_tensor_block(j):
                        tcols += b1 - b0
                    chunks.append((phase, b0, b1, [(j, b0, b1)], tc0))
                    b0 = b1
                j += 1
                continue
            c1 = int(cbase[jend])
            blocks = []
            tc0 = tcols
            for jj in range(j, jend):
                bb0, bb1 = int(cbase[jj]), int(cbase[jj + 1])
                if bb1 > bb0:
                    blocks.append((jj, bb0, bb1))
                    if _is_tensor_block(jj):
                        tcols += bb1 - bb0
            chunks.append((phase, c0, c1, blocks, tc0))
            j = jend
    return chunks, tcols


def _build_diag_a(prep, chunks, tcols):
    """Per-core diag-A array [P, tcols*P] bf16 in chunk order."""
    a = np.zeros((NCORES, P, tcols, P), ml_dtypes.bfloat16)
    rng_p = np.arange(P)
    for c in range(NCORES):
        for (phase, c0, c1, blocks, tc0) in chunks:
            w = prep["w_lo"][c] if phase == 0 else prep["w_hi"][c]
            t = tc0
            for (j, b0, b1) in blocks:
                if not _is_tensor_block(j):
                    continue
                ncol = b1 - b0
                # a[c, p, t+k, p] = w[p, b0+k]
                a[c, rng_p[:, None], t + np.arange(ncol)[None, :], rng_p[:, None]] = (
                    w[:, b0:b1].astype(ml_dtypes.bfloat16)
                )
                t += ncol
    return a.reshape(NCORES, P, tcols * P)


def _build_program(prep, chunks, tcols):
    clo, chi = prep["clo"], prep["chi"]
    SLo, SHi = prep["SLo"], prep["SHi"]

    nc = bacc.Bacc(num_swdge_queues=NQ)

    x_ext = nc.declare_dram_parameter("x", [SHARD, CIN], F32, isOutput=False)
    ilo_ext = nc.declare_dram_parameter("idx_lo", [P, SLo * 8], I16, isOutput=False)
    ihi_ext = nc.declare_dram_parameter("idx_hi", [P, SHi * 8], I16, isOutput=False)
    wlo_ext = nc.declare_dram_parameter("w_lo", [P, SLo], F32, isOutput=False)
    whi_ext = nc.declare_dram_parameter("w_hi", [P, SHi], F32, isOutput=False)
    da_ext = nc.declare_dram_parameter("diag_a", [P, max(tcols, 1) * P], BF16,
                                       isOutput=False)
    dis_ext = nc.declare_dram_parameter("dis", [P, NBLK], F32, isOutput=False)
    wp_ext = [
        nc.declare_dram_parameter(f"Wp{ell}", [CH, CH], F32, isOutput=False)
        for ell in range(3)
    ]
    o_ext = [
        nc.declare_dram_parameter(f"o{ell}", [P, CH], F32, isOutput=False)
        for ell in range(3)
    ]
    lin1_ext = nc.declare_dram_parameter("lin1", [CH, CH], F32, isOutput=False)
    lin2_ext = nc.declare_dram_parameter("lin2", [CH, COUT], F32, isOutput=False)
    l1b_ext = nc.declare_dram_parameter("l1b", [P, CH], F32, isOutput=False)
    l2b_ext = nc.declare_dram_parameter("l2b", [P, COUT], F32, isOutput=False)
    y_ext = nc.declare_dram_parameter("y", [SHARD, COUT], F32, isOutput=True)

    lrelu = mybir.ActivationFunctionType.Lrelu
    actcopy = mybir.ActivationFunctionType.Copy
    ADD = mybir.AluOpType.add
    MULT = mybir.AluOpType.mult

    with TileContext(nc) as tc:
        with (
            tc.tile_pool(name="const", bufs=1) as constp,
            tc.tile_pool(name="gpool", bufs=3) as gp,
            tc.tile_pool(name="apool", bufs=3) as ap_pool,
            tc.tile_pool(name="work", bufs=4) as wk,
            tc.tile_pool(name="psum", bufs=2, space="PSUM") as pp,
            tc.tile_pool(name="psum2", bufs=2, space="PSUM") as pp2,
            tc.tile_pool(name="pacc", bufs=3, space="PSUM") as pacc,
            tc.tile_pool(name="dram", bufs=1, space="DRAM") as dp,
        ):
            ident = constp.tile([P, P], F32)
            make_identity(nc, ident[:])
            idx_lo_t = constp.tile([P, SLo * 8], I16)
            nc.sync.dma_start(out=idx_lo_t[:], in_=ilo_ext[:])
            idx_hi_t = constp.tile([P, SHi * 8], I16)
            nc.sync.dma_start(out=idx_hi_t[:], in_=ihi_ext[:])
            w_lo_t = constp.tile([P, SLo], F32)
            nc.sync.dma_start(out=w_lo_t[:], in_=wlo_ext[:])
            w_hi_t = constp.tile([P, SHi], F32)
            nc.sync.dma_start(out=w_hi_t[:], in_=whi_ext[:])
            dis_t = constp.tile([P, NBLK], F32)
            nc.sync.dma_start(out=dis_t[:], in_=dis_ext[:])
            Wp, o_rep = [], []
            for ell in range(3):
                t = constp.tile([P, CH], F32, name=f"Wp{ell}")
                nc.sync.dma_start(out=t[:], in_=wp_ext[ell][:])
                Wp.append(t)
                t2 = constp.tile([P, CH], F32, name=f"o{ell}")
                nc.sync.dma_start(out=t2[:], in_=o_ext[ell][:])
                o_rep.append(t2)
            lin1_t = constp.tile([P, CH], F32)
            nc.sync.dma_start(out=lin1_t[:], in_=lin1_ext[:])
            lin2_t = constp.tile([P, COUT], F32)
            nc.sync.dma_start(out=lin2_t[:], in_=lin2_ext[:])
            l1b_t = constp.tile([P, CH], F32)
            nc.sync.dma_start(out=l1b_t[:], in_=l1b_ext[:])
            l2b_t = constp.tile([P, COUT], F32)
            nc.sync.dma_start(out=l2b_t[:], in_=l2b_ext[:])

            h = constp.tile([P, NBLK * CH], F32)
            for j in range(NBLK):
                nc.sync.dma_start(
                    out=h[:, j * CH : (j + 1) * CH],
                    in_=x_ext[j * P : (j + 1) * P, :],
                )
            accs = constp.tile([P, NBLK * CH], F32)
            tloc = constp.tile([P, NBLK * CH], BF16)

            tb_list = [j for j in range(NBLK) if _is_tensor_block(j)]

            agin_lo, agin_hi, tab_lo, tab_hi = [], [], [], []
            for ell in range(3):
                agin_lo.append(dp.tile([AGL, CH], BF16, name=f"aglo{ell}"))
                agin_hi.append(dp.tile([AGH, CH], BF16, name=f"aghi{ell}"))
                tab_lo.append(
                    dp.tile([NCORES * AGL, CH], BF16, name=f"tlo{ell}",
                            addr_space="Shared")
                )
                tab_hi.append(
                    dp.tile([NCORES * AGH, CH], BF16, name=f"thi{ell}",
                            addr_space="Shared")
                )

            def emit_table(ell, j):
                """tloc[:, j] = dis * (h_j @ Wp[ell]); DMA to agin."""
                htp = pp.tile([P, P], F32, tag="htp")
                nc.tensor.transpose(
                    out=htp[:], in_=h[:, j * CH : (j + 1) * CH], identity=ident[:]
                )
                hts = wk.tile([P, P], F32, tag="hts")
                nc.scalar.activation(out=hts[:], in_=htp[:], func=actcopy)
                zp = pp2.tile([P, CH], F32, tag="zp")
                nc.tensor.matmul(
                    out=zp[:], lhsT=hts[:], rhs=Wp[ell][:], start=True, stop=True
                )
                tsl = tloc[:, j * CH : (j + 1) * CH]
                nc.scalar.activation(
                    out=tsl, in_=zp[:], func=actcopy, scale=dis_t[:, j : j + 1]
                )
                if j < LO_BLOCKS:
                    nc.sync.dma_start(
                        out=agin_lo[ell][j * P : (j + 1) * P, :], in_=tsl
                    )
                if j >= HI_BLK0:
                    r0 = j * P - FLEX0
                    nc.sync.dma_start(out=agin_hi[ell][r0 : r0 + P, :], in_=tsl)

            def emit_ag(ell):
                nc.gpsimd.collective_compute(
                    "AllGather",
                    mybir.AluOpType.bypass,
                    replica_groups=[list(range(NCORES))],
                    ins=[agin_lo[ell][:]],
                    outs=[tab_lo[ell][:]],
                )
                nc.gpsimd.collective_compute(
                    "AllGather",
                    mybir.AluOpType.bypass,
                    replica_groups=[list(range(NCORES))],
                    ins=[agin_hi[ell][:]],
                    outs=[tab_hi[ell][:]],
                )

            qctr = [0]
            # per-(phase, block) matmul column totals, for start/stop flags
            tb_total = {}
            for (phase, c0, c1, blocks, tc0) in chunks:
                for (j, b0, b1) in blocks:
                    if _is_tensor_block(j):
                        tb_total[(phase, j)] = tb_total.get((phase, j), 0) + (b1 - b0)

            def emit_layer(ell):
                slab_state = {}   # j -> True once acc_j initialized
                tb_chain = {}     # (phase, j) -> [psum_tile, done]

                def merge_slab(j, src_ap):
                    """acc_j (+)= src_ap, with fused self+offset on first."""
                    acc_j = accs[:, j * CH : (j + 1) * CH]
                    if j not in slab_state:
                        nc.vector.scalar_tensor_tensor(
                            out=acc_j,
                            in0=tloc[:, j * CH : (j + 1) * CH],
                            scalar=dis_t[:, j : j + 1],
                            in1=src_ap,
                            op0=MULT,
                            op1=ADD,
                        )
                        nc.vector.tensor_tensor(
                            out=acc_j, in0=acc_j, in1=o_rep[ell][:], op=ADD
                        )
                        slab_state[j] = True
                    else:
                        nc.vector.tensor_tensor(
                            out=acc_j, in0=acc_j, in1=src_ap, op=ADD
                        )

                for (phase, c0, c1, blocks, tc0) in chunks:
                    table = tab_lo[ell] if phase == 0 else tab_hi[ell]
                    idx_t = idx_lo_t if phase == 0 else idx_hi_t
                    w_t = w_lo_t if phase == 0 else w_hi_t
                    ncols = c1 - c0
                    g_t = gp.tile([P, MAX_CHUNK * CH], BF16, tag="g")
                    # split into sub-gathers <= ring size (1024 descs) so the
                    # 4 SWDGE queues drain concurrently instead of each
                    # instruction blocking on its own ring drain
                    s0 = 0
                    while s0 < ncols:
                        s1 = min(s0 + SUB_COLS, ncols)
                        nsub = (s1 - s0) * P
                        nc.gpsimd.dma_gather(
                            g_t[:, s0 * CH : s1 * CH].rearrange(
                                "p (s c) -> p s c", c=CH
                            ),
                            table[:],
                            idx_t[:, (c0 + s0) * 8 : (c0 + s1) * 8],
                            nsub,
                            nsub,
                            CH,
                            queue_num=qctr[0] % NQ,
                            single_packet=False,
                        )
                        qctr[0] += 1
                        s0 = s1
                    # how many tensor cols in this chunk?
                    t_ncols = sum(
                        b1 - b0 for (j, b0, b1) in blocks if _is_tensor_block(j)
                    )
                    if t_ncols:
                        a_t = ap_pool.tile([P, MAX_CHUNK * P], BF16, tag="a")
                        nc.sync.dma_start(
                            out=a_t[:, : t_ncols * P],
                            in_=da_ext[:, tc0 * P : (tc0 + t_ncols) * P],
                        )
                    t_off = 0
                    for (j, b0, b1) in blocks:
                        W = b1 - b0
                        goff = (b0 - c0) * CH
                        if _is_tensor_block(j):
                            key = (phase, j)
                            if key not in tb_chain:
                                tb_chain[key] = [pacc.tile([P, CH], F32, tag="pa", name="pa"), 0]
                            pt, done = tb_chain[key]
                            total = tb_total[key]
                            for k in range(W):
                                nc.tensor.matmul(
                                    out=pt[:],
                                    lhsT=a_t[:, (t_off + k) * P : (t_off + k + 1) * P],
                                    rhs=g_t[:, goff + k * CH : goff + (k + 1) * CH],
                                    start=(done == 0),
                                    stop=(done == total - 1),
                                )
                                done += 1
                            tb_chain[key][1] = done
                            if done == total:
                                merge_slab(j, pt[:])
                            t_off += W
                        else:
                            sl = g_t[:, goff : goff + W * CH]
                            wb = (
                                w_t[:, b0:b1]
                                .unsqueeze(2)
                                .broadcast_to([P, W, CH])
                            )
                            nc.vector.tensor_tensor(out=sl, in0=sl, in1=wb, op=MULT)
                            n = W
                            while n > 1:
                                if n % 2 == 1:
                                    nc.vector.tensor_tensor(
                                        out=sl[:, 0:CH],
                                        in0=sl[:, 0:CH],
                                        in1=sl[:, (n - 1) * CH : n * CH],
                                        op=ADD,
                                    )
                                    n -= 1
                                hh = n // 2
                                nc.vector.tensor_tensor(
                                    out=sl[:, : hh * CH],
                                    in0=sl[:, : hh * CH],
                                    in1=sl[:, hh * CH : 2 * hh * CH],
                                    op=ADD,
                                )
                                n = hh
                            merge_slab(j, sl[:, 0:CH])

                # epilogues
                for j in range(NBLK):
                    acc_j = accs[:, j * CH : (j + 1) * CH]
                    h_j = h[:, j * CH : (j + 1) * CH]
                    if j not in slab_state:
                        # no edge slots at all: acc = tloc*dis + o
                        nc.vector.scalar_tensor_tensor(
                            out=acc_j,
                            in0=tloc[:, j * CH : (j + 1) * CH],
                            scalar=dis_t[:, j : j + 1],
                            in1=o_rep[ell][:],
                            op0=MULT,
                            op1=ADD,
                        )
                    if ell >= 1:
                        nc.vector.tensor_tensor(
                            out=acc_j, in0=acc_j, in1=h_j, op=ADD
                        )
                    nc.scalar.activation(out=h_j, in_=acc_j, func=lrelu, alpha=0.01)
                    if ell < 2:
                        emit_table(ell + 1, j)
                    else:
                        emit_head(j)

            def emit_head(j):
                h_j = h[:, j * CH : (j + 1) * CH]
                htp = pp.tile([P, P], F32, tag="htp")
                nc.tensor.transpose(out=htp[:], in_=h_j, identity=ident[:])
                hts = wk.tile([P, P], F32, tag="hts")
                nc.scalar.activation(out=hts[:], in_=htp[:], func=actcopy)
                z1p = pp2.tile([P, CH], F32, tag="zp")
                nc.tensor.matmul(
                    out=z1p[:], lhsT=hts[:], rhs=lin1_t[:], start=True, stop=True
                )
                z1 = wk.tile([P, CH], F32, tag="z1")
                nc.vector.tensor_tensor(out=z1[:], in0=z1p[:], in1=l1b_t[:], op=ADD)
                nc.scalar.activation(out=z1[:], in_=z1[:], func=lrelu, alpha=0.01)
                z1tp = pp.tile([P, P], F32, tag="htp")
                nc.tensor.transpose(out=z1tp[:], in_=z1[:], identity=ident[:])
                z1ts = wk.tile([P, P], F32, tag="hts")
                nc.scalar.activation(out=z1ts[:], in_=z1tp[:], func=actcopy)
                z2p = pp2.tile([P, CH], F32, tag="zp")
                nc.tensor.matmul(
                    out=z2p[:, :COUT], lhsT=z1ts[:], rhs=lin2_t[:], start=True,
                    stop=True,
                )
                yt = wk.tile([P, COUT], F32, tag="yt")
                nc.vector.tensor_tensor(
                    out=yt[:], in0=z2p[:, :COUT], in1=l2b_t[:], op=ADD
                )
                nc.sync.dma_start(out=y_ext[j * P : (j + 1) * P, :], in_=yt[:])

            # ---- schedule ----
            for j in range(NBLK):
                emit_table(0, j)
            emit_ag(0)
            for ell in range(3):
                emit_layer(ell)
                if ell < 2:
                    emit_ag(ell + 1)

    nc.compile()
    return nc


def kernel(**inputs):
    x = np.asarray(inputs["x"], np.float32)
    edge_index = np.asarray(inputs["edge_index"], np.int64)
    edge_weights = np.asarray(inputs["edge_weights"], np.float32)

    prep = _host_prep(x, edge_index, edge_weights)
    folded = _fold_params(inputs)
    chunks, tcols = _plan(prep)
    diag_a = _build_diag_a(prep, chunks, tcols)

    nc = _build_program(prep, chunks, tcols)

    rep = lambda v: np.tile(np.asarray(v, np.float32)[None, :], (P, 1))
    in_maps = []
    for c in range(NCORES):
        m = {
            "x": prep["x_sh"][c],
            "idx_lo": prep["idx_lo"][c],
            "idx_hi": prep["idx_hi"][c],
            "w_lo": prep["w_lo"][c],
            "w_hi": prep["w_hi"][c],
            "diag_a": diag_a[c] if tcols else np.zeros((P, P), ml_dtypes.bfloat16),
            "dis": prep["dis_sh"][c],
            "lin1": np.asarray(inputs["lin1_w"], np.float32),
            "lin2": np.asarray(inputs["lin2_w"], np.float32),
            "l1b": rep(inputs["lin1_b"]),
            "l2b": rep(inputs["lin2_b"]),
        }
        for ell in range(3):
            m[f"Wp{ell}"] = folded[f"Wp{ell}"]
            m[f"o{ell}"] = rep(folded[f"o{ell}"])
        in_maps.append(m)

    res = run_bass_kernel_spmd(nc, in_maps, core_ids=list(range(NCORES)))
    global _last_results
    _last_results = res

    out = np.empty((N, COUT), np.float32)
    order = prep["order"]
    for c in range(NCORES):
        ranks = np.arange(c, N, NCORES)
        out[order[ranks]] = res.results[c]["y"][: len(ranks)]
    return out
